# revision 29
# baseline (speedup 1.0000x reference)
"""Trainium2 Bass kernel for nn_MultiHeadAttention_38027640439053.

Reference computation (per batch b of 8, one NeuronCore each):
    data = X.reshape(n, 16, 64)
    q/k/v = data @ W{q,k,v}.T          (per-head shared 64x64 weights)
    scores = (q @ k.T per head) / 32
    attn = softmax(scores, axis=k)
    Y = (attn @ v).reshape(n, 1024) @ Wo.T + bo

Strategy (batch-parallel over 8 cores, zero collectives, bf16 compute):
  - X is converted to bf16 on the host; each pair-of-heads column slab
    loads TRANSPOSED straight from DRAM via the XBAR DMA transpose
    (dma_start_transpose) -> zero PE transposes.
  - Q and K projections are FUSED: scores = X A X^T with A = Wq^T Wk
    (shared by all 16 heads), so one projected operand GT = A2^T @ XT
    replaces both Q and K; the score matmuls' stationary side is raw
    XT.  The two heads of a pair occupy PE row groups 0-63 / 64-127
    and their score MMs are emitted chunk-major so they overlap in
    the array on HW.
  - exp runs on ScalarE from PSUM (1/32 scale folded in, bf16 out);
    4 of every 16 score tiles instead use a Schraudolph bit-trick exp
    on the otherwise-idle DVE (j = int16(s*A+B) bits read as bf16),
    balancing the two engines.  End-to-end rel err 8.8e-3 (gate 2e-2).
  - P@V transposed with a ones-augmented V; pvps row 64 is the softmax
    denominator.  1/D comes from DVE reciprocal straight out of PSUM
    into rows 0/64 of a persistent [65,N] tile; a [65->128] selector
    matmul broadcasts it and one DVE multiply normalizes each pair.
  - Wo^T (bf16) and bias-broadcast preload early; output projection
    accumulates 8 pair-chunks per n-tile in PSUM and the bias is added
    by the DVE during the PSUM->SBUF drain.
  - Pipelining: pair p's ACT/DVE-paced score/exp loop absorbs pair
    p-1's PV (4-MM bundles at every k-tile), pair p+1's loads and
    projections, and (for the last pair) its own eager PV; pair 0's
    loop hosts its own V projection and the bias-broadcast build.
    Two score tiles per pair borrow an idle mps PSUM slot (third exp
    buffer), and the first output-projection accumulator plus the last
    pair's second PV head borrow the score PSUM slots so the tail
    starts without waiting on the finish/normalize DVE chain.
    Simulated span 171.4 us single-shot / 167.3 us marginal per rep
    (baseline kernel simmed 232.5 us); HW rel err 8.78e-3.
"""

import numpy as np
import ml_dtypes

import concourse.bacc as bacc
import concourse.mybir as mybir
import concourse.tile as tile
from concourse.bass_utils import run_bass_kernel_spmd

F32 = mybir.dt.float32
BF16 = mybir.dt.bfloat16
I16 = mybir.dt.int16

EXP = mybir.ActivationFunctionType.Exp

# (ktile, head) score tiles whose exp runs on the DVE via the Schraudolph
# bit trick (j = int16(s*A + B); bits reinterpreted as bf16 ~= 2^(s*log2e)).
# Balances the ScalarE exp load against idle DVE capacity; each DVE tile
# adds ~3% sawtooth error to its attention weights (end-to-end rel err
# measured 8.4e-3 vs the 2e-2 gate).
DVE_EXP = frozenset({(1, 0), (3, 1), (5, 0), (7, 1)})
SCHR_A = 128.0 * float(np.log2(np.e))   # per unit *scaled* score
SCHR_B = 128.0 * (127.0 - 0.0434)


def emit_body(tc, nc, aps, N, EMB, NH, rep):
    NPAIR = NH // 2
    NT = N // 128        # n tiles (rows of X / q tiles)
    KT = N // 128        # k tiles
    assert EMB == NPAIR * 128
    scale = 1.0 / float(np.sqrt(EMB))
    qch = [(s, min(512, N - s)) for s in range(0, N, 512)]
    ech = [(s, min(512, EMB - s)) for s in range(0, EMB, 512)]

    X_d, A2_d, Wv2_d, WoT_d, bo_d, sel_d, ones_d, Y_d = aps

    with (
        tc.tile_pool(name=f"consts{rep}", bufs=1) as consts,
        tc.tile_pool(name=f"xtp{rep}", bufs=3) as xtp,
        tc.tile_pool(name=f"gtp{rep}", bufs=2) as gtp,
        tc.tile_pool(name=f"vp{rep}", bufs=3) as vp,
        tc.tile_pool(name=f"ptp{rep}", bufs=3) as ptp,
        tc.tile_pool(name=f"ytp{rep}", bufs=NPAIR) as ytp,
        tc.tile_pool(name=f"rdp{rep}", bufs=1) as rdp,
        tc.tile_pool(name=f"osbp{rep}", bufs=2) as osbp,
        tc.tile_pool(name=f"stps{rep}", bufs=2, space="PSUM") as stps,
        tc.tile_pool(name=f"mps{rep}", bufs=2, space="PSUM") as mps,
    ):
        # ---- constants needed immediately: ACT hwdge queue, so the SP
        # queue starts xt0's DMA-transpose at t=0 ----
        a2 = consts.tile([128, 128], BF16, name="a2", tag="a2")
        nc.scalar.dma_start(out=a2[:], in_=A2_d[:])
        wv2 = consts.tile([128, 128], BF16, name="wv2", tag="wv2")
        nc.scalar.dma_start(out=wv2[:], in_=Wv2_d[:])

        # persistent denominator tile: rows 0 / 64 hold 1/D of the current
        # pair's two heads; all other rows stay zero forever.
        ds = rdp.tile([65, N], BF16, name="ds", tag="ds")
        nc.gpsimd.memset(ds[:], 0.0)

        # ---- late-need constants (declared here, DMAs emitted after the
        # pair-0 prologue so they queue behind xt0 on SP) ----
        bo_t = consts.tile([1, EMB], BF16, name="bo_t", tag="bo_t")
        ones_t = consts.tile([1, 128], BF16, name="ones_t", tag="ones_t")
        sel_t = consts.tile([65, 128], BF16, name="sel_t", tag="sel_t")
        wot = consts.tile([128, NPAIR * EMB], BF16, name="wot", tag="wot")
        bobc = consts.tile([128, EMB], BF16, name="bobc", tag="bobc")

        def load_late_consts():
            nc.sync.dma_start(out=sel_t[:], in_=sel_d[:])
            nc.sync.dma_start(out=bo_t[:], in_=bo_d[:])
            nc.sync.dma_start(out=ones_t[:], in_=ones_d[:])
            nc.sync.dma_start(
                out=wot[:].rearrange("p (c e) -> p c e", e=EMB),
                in_=WoT_d[:].rearrange("(c p) e -> p c e", p=128))

        # ---- per-pair helpers ----
        xts = {}

        def load_xt(p):
            xt = xtp.tile([128, N], BF16, name=f"xt{p}", tag="xt")
            nc.sync.dma_start_transpose(
                out=xt[:], in_=X_d[:, p * 128:(p + 1) * 128])
            xts[p] = xt

        def proj_gt(p):
            xt = xts[p]
            gps = mps.tile([128, N], F32, name=f"gps{p}", tag="m")
            for (s, w) in qch:
                nc.tensor.matmul(gps[:, s:s + w], a2[:], xt[:, s:s + w])
            gt = gtp.tile([128, N], BF16, name=f"gt{p}", tag="gt")
            nc.vector.tensor_copy(gt[:], gps[:])
            return gt

        def proj_v(p):
            xt = xts[p]
            vps = mps.tile([128, N], F32, name=f"vps{p}", tag="m")
            for i in range(NT):
                nc.tensor.matmul(vps[:, i * 128:(i + 1) * 128],
                                 xt[:, i * 128:(i + 1) * 128], wv2[:])
            vslab = vp.tile([128, KT * 130], BF16, name=f"vslab{p}", tag="v")
            v4 = vslab[:].rearrange("p (j k c) -> p j k c", k=2, c=65)
            vs4 = vps[:].rearrange("p (j k c) -> p j k c", k=2, c=64)
            nc.vector.tensor_copy(v4[:, :, :, 0:64], vs4[:])
            nc.gpsimd.memset(v4[:, :, :, 64:65], 1.0)
            return vslab

        def st_exp(p, ktile, gt, pt):
            """Transposed scores + exp for one k-tile, both heads.

            Score matmuls are emitted chunk-major (h0c0, h1c0, h0c1, h1c1)
            so consecutive MMs target alternating PE row groups (partition
            bases 0/64) and overlap in the array on HW.
            """
            xt = xts[p]
            sts = []
            for head in (0, 1):
                # k-tiles 2 and 5 (head 0) borrow an idle mps slot: a third
                # score buffer that breaks the 2-slot PE<->exp ping-pong.
                if head == 0 and ktile in (2, 5):
                    sts.append(mps.tile([128, N], F32,
                                        name=f"st{p}_{ktile}_{head}", tag="m"))
                else:
                    sts.append(stps.tile([128, N], F32,
                                         name=f"st{p}_{ktile}_{head}",
                                         tag="st"))
            for (s, w) in qch:
                for head in (0, 1):
                    r0 = head * 64
                    nc.tensor.matmul(
                        sts[head][:, s:s + w],
                        xt[r0:r0 + 64, ktile * 128:(ktile + 1) * 128],
                        gt[r0:r0 + 64, s:s + w],
                    )
            for head in (0, 1):
                dst = pt[:, (ktile * 2 + head) * N:(ktile * 2 + head + 1) * N]
                if (ktile, head) in DVE_EXP:
                    with nc.allow_low_precision(reason="schraudolph exp"):
                        nc.vector.tensor_scalar(
                            dst.bitcast(I16), sts[head][:],
                            SCHR_A * scale, SCHR_B,
                            mybir.AluOpType.mult, mybir.AluOpType.add)
                else:
                    nc.scalar.activation(dst, sts[head][:], EXP, scale=scale)

        pv_state = {}

        def pv_q(p, head, qk, vslab, pt, pool=None):
            """4 accumulating matmuls: k-tiles [qk*2, qk*2+2)."""
            if qk == 0:
                pv_state[(p, head)] = (pool or mps).tile(
                    [65, N], F32, name=f"pvps{p}_{head}",
                    tag="st" if pool is not None else "m")
            pvps = pv_state[(p, head)]
            k0, k1 = qk * 2, qk * 2 + 2
            for ktile in range(k0, k1):
                lhs = vslab[:, ktile * 130 + head * 65:
                            ktile * 130 + head * 65 + 65]
                base = (ktile * 2 + head) * N
                for (s, w) in qch:
                    nc.tensor.matmul(
                        pvps[:, s:s + w], lhs,
                        pt[:, base + s:base + s + w],
                        start=(ktile == 0), stop=(ktile == KT - 1),
                    )

        def finish_head(p, head, yt):
            pvps = pv_state.pop((p, head))
            nc.vector.tensor_copy(yt[head * 64:head * 64 + 64, :],
                                  pvps[0:64, :])
            with nc.allow_low_precision(reason="bf16 softmax denom"):
                nc.vector.reciprocal(ds[head * 64:head * 64 + 1, :],
                                     pvps[64:65, :])

        def make_bobc():
            # broadcast bo across all 128 partitions once; the tail adds it
            # during the PSUM->SBUF drain instead of 2 matmuls per n-tile.
            bps = mps.tile([128, EMB], F32, name="bobc_ps", tag="m")
            for (s, w) in ech:
                nc.tensor.matmul(bps[:, s:s + w], ones_t[:], bo_t[:, s:s + w])
            nc.vector.tensor_copy(bobc[:], bps[:])

        def bcast_mul(p, yt):
            bps = mps.tile([128, N], F32, name=f"bps{p}", tag="m")
            for (s, w) in qch:
                nc.tensor.matmul(bps[:, s:s + w], sel_t[:], ds[:, s:s + w])
            nc.vector.tensor_mul(yt[:], yt[:], bps[:])

        # ---------------- pipelined pair loop ----------------
        yts = []
        pts = {}
        vslabs = {}

        load_xt(0)
        cur_gt = proj_gt(0)
        load_late_consts()
        nxt = {}
        for p in range(NPAIR):
            pt = ptp.tile([128, KT * 2 * N], BF16, name=f"pt{p}", tag="pt")
            pts[p] = pt
            yts.append(ytp.tile([128, N], BF16, name=f"yt{p}", tag="yt"))

            sched = {k: [] for k in range(KT)}
            if p > 0:
                po, vo, pp = p - 1, vslabs[p - 1], pts[p - 1]
                yo = yts[p - 1]
                # 4-MM PV bundles at every k-tile position: keeps the PE
                # fed in each inter-exp window instead of in large bursts.
                for qk in range(4):
                    sched[qk].append(
                        lambda qk=qk: pv_q(po, 0, qk, vo, pp))
                sched[3].append(lambda: finish_head(po, 0, yo))
                for qk in range(4):
                    sched[3 + qk].append(
                        lambda qk=qk: pv_q(po, 1, qk, vo, pp))
                sched[6].append(lambda: finish_head(po, 1, yo))
                sched[7].append(lambda: bcast_mul(po, yo))
            if p + 1 < NPAIR:
                pn = p + 1
                tasks = [
                    lambda: load_xt(pn),
                    lambda: nxt.__setitem__("gt", proj_gt(pn)),
                    lambda: vslabs.__setitem__(pn, proj_v(pn)),
                ]
                for j, pos in enumerate((0, KT - 7, KT - 4)):
                    sched[max(0, pos)].append(tasks[j])
                if p == 0:
                    # pair 0 has no previous-pair PV to absorb: fill its
                    # exp-paced loop with its own V projection and the
                    # bias-broadcast build instead.
                    sched[2].append(
                        lambda: vslabs.__setitem__(0, proj_v(0)))
                    sched[5].append(make_bobc)
            else:
                # eager PV for the last pair: k-tiles 0-3 of head 0 only —
                # their pt slices are already emitted by then.
                sched[KT // 2].append(
                    lambda: (pv_q(p, 0, 0, vslabs[p], pts[p]),
                             pv_q(p, 0, 1, vslabs[p], pts[p])))
            for ktile in range(KT):
                for t in sched[ktile]:
                    t()
                st_exp(p, ktile, cur_gt, pt)
            if p - 1 >= 0:
                del vslabs[p - 1], pts[p - 1]
            if p + 1 < NPAIR:
                cur_gt = nxt["gt"]

        # ---------------- tail: last pair's PV + outproj ----
        last = NPAIR - 1
        pv_q(last, 0, 2, vslabs[last], pts[last])
        pv_q(last, 0, 3, vslabs[last], pts[last])
        finish_head(last, 0, yts[last])
        for qk in range(4):
            pv_q(last, 1, qk, vslabs[last], pts[last], pool=stps)
        finish_head(last, 1, yts[last])
        bcast_mul(last, yts[last])

        for i in range(NT):
            pool = stps if i == 0 else mps
            ops = pool.tile([128, EMB], F32, name=f"ops{i}",
                            tag="st" if i == 0 else "m")
            for p in range(NPAIR):
                for (s, w) in ech:
                    nc.tensor.matmul(
                        ops[:, s:s + w],
                        yts[p][:, i * 128:(i + 1) * 128],
                        wot[:, p * EMB + s:p * EMB + s + w],
                        start=(p == 0), stop=(p == NPAIR - 1),
                    )
            osb = osbp.tile([128, EMB], F32, name=f"osb{i}", tag="osb")
            # drain in halves so the last add overlaps the last Y DMA
            for (s, w) in ech:
                nc.vector.tensor_add(osb[:, s:s + w], ops[:, s:s + w],
                                     bobc[:, s:s + w])
                nc.sync.dma_start(out=Y_d[i * 128:(i + 1) * 128, s:s + w],
                                  in_=osb[:, s:s + w])


def build_program(N=1024, EMB=1024, NH=16, n_cores=8, repeat=1,
                  trace_sim=False):
    nc = bacc.Bacc("TRN2", target_bir_lowering=False, debug=False,
                   num_devices=n_cores)
    aps = (
        nc.dram_tensor("X", [N, EMB], BF16, kind="ExternalInput").ap(),
        nc.dram_tensor("A2", [128, 128], BF16, kind="ExternalInput").ap(),
        nc.dram_tensor("Wv2", [128, 128], BF16, kind="ExternalInput").ap(),
        nc.dram_tensor("WoT", [EMB, EMB], BF16, kind="ExternalInput").ap(),
        nc.dram_tensor("bo", [1, EMB], BF16, kind="ExternalInput").ap(),
        nc.dram_tensor("sel", [65, 128], BF16, kind="ExternalInput").ap(),
        nc.dram_tensor("ones", [1, 128], BF16, kind="ExternalInput").ap(),
        nc.dram_tensor("Y", [N, EMB], F32, kind="ExternalOutput").ap(),
    )
    with tile.TileContext(nc, trace_sim=trace_sim) as tc:
        for rep in range(repeat):
            emit_body(tc, nc, aps, N, EMB, NH, rep)
    nc.compile()
    return nc


def host_consts(Wq, Wk, Wv, Wo, bo, NH=16):
    EMB = NH * 64
    bf = ml_dtypes.bfloat16

    A = np.asarray(Wq, np.float32).T @ np.asarray(Wk, np.float32)

    def blk2(B):
        out = np.zeros((128, 128), np.float32)
        out[0:64, 0:64] = B
        out[64:128, 64:128] = B
        return out

    # selector: row 0 -> output partitions 0..63, row 64 -> 64..127
    sel = np.zeros((65, 128), np.float32)
    sel[0, 0:64] = 1.0
    sel[64, 64:128] = 1.0
    return {
        "A2": blk2(A).astype(bf),
        "Wv2": blk2(np.asarray(Wv, np.float32).T).astype(bf),
        "WoT": np.ascontiguousarray(
            np.asarray(Wo, np.float32).T).astype(bf),
        "bo": np.asarray(bo, np.float32).reshape(1, EMB).astype(bf),
        "sel": sel.astype(bf),
        "ones": np.ones((1, 128), np.float32).astype(bf),
    }


def stage_x(X_core):
    """Convert one core's [N, EMB] fp32 activation slab to bf16."""
    return np.ascontiguousarray(
        np.asarray(X_core, np.float32).astype(ml_dtypes.bfloat16))


_NC_CACHE = {}


def kernel(X, Wq, Wk, Wv, Wo, bo):
    X = np.asarray(X, np.float32)
    B, N, EMB = X.shape
    NH = EMB // 64
    key = (N, EMB, NH, B)
    if key not in _NC_CACHE:
        _NC_CACHE[key] = build_program(N=N, EMB=EMB, NH=NH, n_cores=B)
    nc = _NC_CACHE[key]
    consts = host_consts(Wq, Wk, Wv, Wo, bo, NH=NH)
    in_maps = [dict(consts, X=stage_x(X[c])) for c in range(B)]
    res = run_bass_kernel_spmd(nc, in_maps, list(range(B)))
    return np.stack([res.results[c]["Y"] for c in range(B)], axis=0)


if __name__ == "__main__":
    rng = np.random.default_rng(0)
    B, N, EMB, NH = 8, 1024, 1024, 16
    X = rng.standard_normal((B, N, EMB), dtype=np.float32)
    Wq = (rng.standard_normal((64, 64), dtype=np.float32) / 8)
    Wk = (rng.standard_normal((64, 64), dtype=np.float32) / 8)
    Wv = (rng.standard_normal((64, 64), dtype=np.float32) / 8)
    Wo = (rng.standard_normal((EMB, EMB), dtype=np.float32) / 32)
    bo = np.zeros(EMB, np.float32)
    Y = kernel(X=X, Wq=Wq, Wk=Wk, Wv=Wv, Wo=Wo, bo=bo)
    print("OK", Y.shape, Y.dtype)


# revision 37
# speedup vs baseline: 1.4672x; 1.4672x over previous
"""Trainium2 Bass kernel for nn_MultiHeadAttention_38027640439053.

Reference computation (per batch b of 8, one NeuronCore each):
    data = X.reshape(n, 16, 64)
    q/k/v = data @ W{q,k,v}.T          (per-head shared 64x64 weights)
    scores = (q @ k.T per head) / 32
    attn = softmax(scores, axis=k)
    Y = (attn @ v).reshape(n, 1024) @ Wo.T + bo

Strategy (batch-parallel over 8 cores, zero collectives, bf16 compute):
  - X is converted to bf16 on the host; each pair-of-heads column slab
    loads TRANSPOSED straight from DRAM via the XBAR DMA transpose
    (dma_start_transpose) -> zero PE transposes.
  - Q and K projections are FUSED: scores = X A X^T with A = Wq^T Wk
    (shared by all 16 heads), so one projected operand GT = A2^T @ XT
    replaces both Q and K; the score matmuls' stationary side is raw
    XT.  The two heads of a pair occupy PE row groups 0-63 / 64-127
    and their score MMs are emitted chunk-major so they overlap in
    the array on HW.
  - exp runs on ScalarE from PSUM (1/32 scale folded in, bf16 out);
    4 of every 16 score tiles instead use a Schraudolph bit-trick exp
    on the otherwise-idle DVE (j = int16(s*A+B) bits read as bf16),
    balancing the two engines.  End-to-end rel err 8.8e-3 (gate 2e-2).
  - P@V transposed with a ones-augmented V; pvps row 64 is the softmax
    denominator.  1/D comes from DVE reciprocal straight out of PSUM
    into rows 0/64 of a persistent [65,N] tile; a [65->128] selector
    matmul broadcasts it and one DVE multiply normalizes each pair.
  - Wo^T (bf16) and bias-broadcast preload early; output projection
    accumulates 8 pair-chunks per n-tile in PSUM and the bias is added
    by the DVE during the PSUM->SBUF drain.
  - Pipelining: pair p's ACT/DVE-paced score/exp loop absorbs pair
    p-1's PV (4-MM bundles at every k-tile), pair p+1's loads and
    projections, and (for the last pair) its own eager PV; pair 0's
    loop hosts its own V projection and the bias-broadcast build.
    Two score tiles per pair borrow an idle mps PSUM slot (third exp
    buffer), and the first output-projection accumulator plus the last
    pair's second PV head borrow the score PSUM slots so the tail
    starts without waiting on the finish/normalize DVE chain.
    The h1 PV bundles sit at k-tiles 3/4/6/7, feeding the otherwise
    fill-starved end of each pair's exp stream.
    Simulated span 171.3 us single-shot / 167.2 us marginal per rep
    (baseline kernel simmed 232.5 us); HW rel err 8.78e-3.
"""

import numpy as np
import ml_dtypes

import concourse.bacc as bacc
import concourse.mybir as mybir
import concourse.tile as tile
from concourse.bass_utils import run_bass_kernel_spmd

F32 = mybir.dt.float32
BF16 = mybir.dt.bfloat16
I16 = mybir.dt.int16

EXP = mybir.ActivationFunctionType.Exp

# (ktile, head) score tiles whose exp runs on the DVE via the Schraudolph
# bit trick (j = int16(s*A + B); bits reinterpreted as bf16 ~= 2^(s*log2e)).
# Balances the ScalarE exp load against idle DVE capacity; each DVE tile
# adds ~3% sawtooth error to its attention weights (end-to-end rel err
# measured 8.4e-3 vs the 2e-2 gate).
DVE_EXP = frozenset({(1, 0), (3, 1), (5, 0), (7, 1)})
SCHR_A = 128.0 * float(np.log2(np.e))   # per unit *scaled* score
SCHR_B = 128.0 * (127.0 - 0.0434)


def emit_body(tc, nc, aps, N, EMB, NH, rep):
    NPAIR = NH // 2
    NT = N // 128        # n tiles (rows of X / q tiles)
    KT = N // 128        # k tiles
    assert EMB == NPAIR * 128
    scale = 1.0 / float(np.sqrt(EMB))
    qch = [(s, min(512, N - s)) for s in range(0, N, 512)]
    ech = [(s, min(512, EMB - s)) for s in range(0, EMB, 512)]

    X_d, A2_d, Wv2_d, WoT_d, bo_d, sel_d, ones_d, Y_d = aps

    with (
        tc.tile_pool(name=f"consts{rep}", bufs=1) as consts,
        tc.tile_pool(name=f"xtp{rep}", bufs=3) as xtp,
        tc.tile_pool(name=f"gtp{rep}", bufs=2) as gtp,
        tc.tile_pool(name=f"vp{rep}", bufs=3) as vp,
        tc.tile_pool(name=f"ptp{rep}", bufs=3) as ptp,
        tc.tile_pool(name=f"ytp{rep}", bufs=NPAIR) as ytp,
        tc.tile_pool(name=f"rdp{rep}", bufs=1) as rdp,
        tc.tile_pool(name=f"osbp{rep}", bufs=2) as osbp,
        tc.tile_pool(name=f"stps{rep}", bufs=2, space="PSUM") as stps,
        tc.tile_pool(name=f"mps{rep}", bufs=2, space="PSUM") as mps,
    ):
        # ---- constants needed immediately: ACT hwdge queue, so the SP
        # queue starts xt0's DMA-transpose at t=0 ----
        a2 = consts.tile([128, 128], BF16, name="a2", tag="a2")
        nc.scalar.dma_start(out=a2[:], in_=A2_d[:])
        wv2 = consts.tile([128, 128], BF16, name="wv2", tag="wv2")
        nc.scalar.dma_start(out=wv2[:], in_=Wv2_d[:])

        # persistent denominator tile: rows 0 / 64 hold 1/D of the current
        # pair's two heads; all other rows stay zero forever.
        ds = rdp.tile([65, N], BF16, name="ds", tag="ds")
        nc.gpsimd.memset(ds[:], 0.0)

        # ---- late-need constants (declared here, DMAs emitted after the
        # pair-0 prologue so they queue behind xt0 on SP) ----
        bo_t = consts.tile([1, EMB], BF16, name="bo_t", tag="bo_t")
        ones_t = consts.tile([1, 128], BF16, name="ones_t", tag="ones_t")
        sel_t = consts.tile([65, 128], BF16, name="sel_t", tag="sel_t")
        wot = consts.tile([128, NPAIR * EMB], BF16, name="wot", tag="wot")
        bobc = consts.tile([128, EMB], BF16, name="bobc", tag="bobc")

        def load_late_consts():
            nc.sync.dma_start(out=sel_t[:], in_=sel_d[:])
            nc.sync.dma_start(out=bo_t[:], in_=bo_d[:])
            nc.sync.dma_start(out=ones_t[:], in_=ones_d[:])
            nc.sync.dma_start(
                out=wot[:].rearrange("p (c e) -> p c e", e=EMB),
                in_=WoT_d[:].rearrange("(c p) e -> p c e", p=128))

        # ---- per-pair helpers ----
        xts = {}

        def load_xt(p):
            xt = xtp.tile([128, N], BF16, name=f"xt{p}", tag="xt")
            nc.sync.dma_start_transpose(
                out=xt[:], in_=X_d[:, p * 128:(p + 1) * 128])
            xts[p] = xt

        def proj_gt(p):
            xt = xts[p]
            gps = mps.tile([128, N], F32, name=f"gps{p}", tag="m")
            for (s, w) in qch:
                nc.tensor.matmul(gps[:, s:s + w], a2[:], xt[:, s:s + w])
            gt = gtp.tile([128, N], BF16, name=f"gt{p}", tag="gt")
            nc.vector.tensor_copy(gt[:], gps[:])
            return gt

        def proj_v(p):
            xt = xts[p]
            vps = mps.tile([128, N], F32, name=f"vps{p}", tag="m")
            for i in range(NT):
                nc.tensor.matmul(vps[:, i * 128:(i + 1) * 128],
                                 xt[:, i * 128:(i + 1) * 128], wv2[:])
            vslab = vp.tile([128, KT * 130], BF16, name=f"vslab{p}", tag="v")
            v4 = vslab[:].rearrange("p (j k c) -> p j k c", k=2, c=65)
            vs4 = vps[:].rearrange("p (j k c) -> p j k c", k=2, c=64)
            nc.vector.tensor_copy(v4[:, :, :, 0:64], vs4[:])
            nc.gpsimd.memset(v4[:, :, :, 64:65], 1.0)
            return vslab

        def st_exp(p, ktile, gt, pt):
            """Transposed scores + exp for one k-tile, both heads.

            Score matmuls are emitted chunk-major (h0c0, h1c0, h0c1, h1c1)
            so consecutive MMs target alternating PE row groups (partition
            bases 0/64) and overlap in the array on HW.
            """
            xt = xts[p]
            sts = []
            for head in (0, 1):
                # k-tiles 2 and 5 (head 0) borrow an idle mps slot: a third
                # score buffer that breaks the 2-slot PE<->exp ping-pong.
                if head == 0 and ktile in (2, 5):
                    sts.append(mps.tile([128, N], F32,
                                        name=f"st{p}_{ktile}_{head}", tag="m"))
                else:
                    sts.append(stps.tile([128, N], F32,
                                         name=f"st{p}_{ktile}_{head}",
                                         tag="st"))
            for (s, w) in qch:
                for head in (0, 1):
                    r0 = head * 64
                    nc.tensor.matmul(
                        sts[head][:, s:s + w],
                        xt[r0:r0 + 64, ktile * 128:(ktile + 1) * 128],
                        gt[r0:r0 + 64, s:s + w],
                    )
            for head in (0, 1):
                dst = pt[:, (ktile * 2 + head) * N:(ktile * 2 + head + 1) * N]
                if (ktile, head) in DVE_EXP:
                    with nc.allow_low_precision(reason="schraudolph exp"):
                        nc.vector.tensor_scalar(
                            dst.bitcast(I16), sts[head][:],
                            SCHR_A * scale, SCHR_B,
                            mybir.AluOpType.mult, mybir.AluOpType.add)
                else:
                    nc.scalar.activation(dst, sts[head][:], EXP, scale=scale)

        pv_state = {}

        def pv_q(p, head, qk, vslab, pt, pool=None):
            """4 accumulating matmuls: k-tiles [qk*2, qk*2+2)."""
            if qk == 0:
                pv_state[(p, head)] = (pool or mps).tile(
                    [65, N], F32, name=f"pvps{p}_{head}",
                    tag="st" if pool is not None else "m")
            pvps = pv_state[(p, head)]
            k0, k1 = qk * 2, qk * 2 + 2
            for ktile in range(k0, k1):
                lhs = vslab[:, ktile * 130 + head * 65:
                            ktile * 130 + head * 65 + 65]
                base = (ktile * 2 + head) * N
                for (s, w) in qch:
                    nc.tensor.matmul(
                        pvps[:, s:s + w], lhs,
                        pt[:, base + s:base + s + w],
                        start=(ktile == 0), stop=(ktile == KT - 1),
                    )

        def finish_head(p, head, yt):
            pvps = pv_state.pop((p, head))
            nc.vector.tensor_copy(yt[head * 64:head * 64 + 64, :],
                                  pvps[0:64, :])
            with nc.allow_low_precision(reason="bf16 softmax denom"):
                nc.vector.reciprocal(ds[head * 64:head * 64 + 1, :],
                                     pvps[64:65, :])

        def make_bobc():
            # broadcast bo across all 128 partitions once; the tail adds it
            # during the PSUM->SBUF drain instead of 2 matmuls per n-tile.
            bps = mps.tile([128, EMB], F32, name="bobc_ps", tag="m")
            for (s, w) in ech:
                nc.tensor.matmul(bps[:, s:s + w], ones_t[:], bo_t[:, s:s + w])
            nc.vector.tensor_copy(bobc[:], bps[:])

        def bcast_mul(p, yt):
            bps = mps.tile([128, N], F32, name=f"bps{p}", tag="m")
            for (s, w) in qch:
                nc.tensor.matmul(bps[:, s:s + w], sel_t[:], ds[:, s:s + w])
            nc.vector.tensor_mul(yt[:], yt[:], bps[:])

        # ---------------- pipelined pair loop ----------------
        yts = []
        pts = {}
        vslabs = {}

        load_xt(0)
        cur_gt = proj_gt(0)
        load_late_consts()
        nxt = {}
        for p in range(NPAIR):
            pt = ptp.tile([128, KT * 2 * N], BF16, name=f"pt{p}", tag="pt")
            pts[p] = pt
            yts.append(ytp.tile([128, N], BF16, name=f"yt{p}", tag="yt"))

            sched = {k: [] for k in range(KT)}
            if p > 0:
                po, vo, pp = p - 1, vslabs[p - 1], pts[p - 1]
                yo = yts[p - 1]
                # 4-MM PV bundles at every k-tile position: keeps the PE
                # fed in each inter-exp window instead of in large bursts.
                for qk in range(4):
                    sched[qk].append(
                        lambda qk=qk: pv_q(po, 0, qk, vo, pp))
                sched[3].append(lambda: finish_head(po, 0, yo))
                for qk, pos in enumerate((3, 4, 6, 7)):
                    sched[pos].append(
                        lambda qk=qk: pv_q(po, 1, qk, vo, pp))
                sched[7].append(
                    lambda: (finish_head(po, 1, yo), bcast_mul(po, yo)))
            if p + 1 < NPAIR:
                pn = p + 1
                tasks = [
                    lambda: load_xt(pn),
                    lambda: nxt.__setitem__("gt", proj_gt(pn)),
                    lambda: vslabs.__setitem__(pn, proj_v(pn)),
                ]
                for j, pos in enumerate((0, KT - 7, KT - 4)):
                    sched[max(0, pos)].append(tasks[j])
                if p == 0:
                    # pair 0 has no previous-pair PV to absorb: fill its
                    # exp-paced loop with its own V projection and the
                    # bias-broadcast build instead.
                    sched[2].append(
                        lambda: vslabs.__setitem__(0, proj_v(0)))
                    sched[5].append(make_bobc)
            else:
                # eager PV for the last pair: k-tiles 0-3 of head 0 only —
                # their pt slices are already emitted by then.
                sched[KT // 2].append(
                    lambda: (pv_q(p, 0, 0, vslabs[p], pts[p]),
                             pv_q(p, 0, 1, vslabs[p], pts[p])))
            for ktile in range(KT):
                for t in sched[ktile]:
                    t()
                st_exp(p, ktile, cur_gt, pt)
            if p - 1 >= 0:
                del vslabs[p - 1], pts[p - 1]
            if p + 1 < NPAIR:
                cur_gt = nxt["gt"]

        # ---------------- tail: last pair's PV + outproj ----
        last = NPAIR - 1
        pv_q(last, 0, 2, vslabs[last], pts[last])
        pv_q(last, 0, 3, vslabs[last], pts[last])
        finish_head(last, 0, yts[last])
        for qk in range(4):
            pv_q(last, 1, qk, vslabs[last], pts[last], pool=stps)
        finish_head(last, 1, yts[last])
        bcast_mul(last, yts[last])

        for i in range(NT):
            pool = stps if i == 0 else mps
            ops = pool.tile([128, EMB], F32, name=f"ops{i}",
                            tag="st" if i == 0 else "m")
            for p in range(NPAIR):
                for (s, w) in ech:
                    nc.tensor.matmul(
                        ops[:, s:s + w],
                        yts[p][:, i * 128:(i + 1) * 128],
                        wot[:, p * EMB + s:p * EMB + s + w],
                        start=(p == 0), stop=(p == NPAIR - 1),
                    )
            osb = osbp.tile([128, EMB], F32, name=f"osb{i}", tag="osb")
            # drain in halves so the last add overlaps the last Y DMA
            for (s, w) in ech:
                nc.vector.tensor_add(osb[:, s:s + w], ops[:, s:s + w],
                                     bobc[:, s:s + w])
                nc.sync.dma_start(out=Y_d[i * 128:(i + 1) * 128, s:s + w],
                                  in_=osb[:, s:s + w])


def build_program(N=1024, EMB=1024, NH=16, n_cores=8, repeat=1,
                  trace_sim=False):
    nc = bacc.Bacc("TRN2", target_bir_lowering=False, debug=False,
                   num_devices=n_cores)
    aps = (
        nc.dram_tensor("X", [N, EMB], BF16, kind="ExternalInput").ap(),
        nc.dram_tensor("A2", [128, 128], BF16, kind="ExternalInput").ap(),
        nc.dram_tensor("Wv2", [128, 128], BF16, kind="ExternalInput").ap(),
        nc.dram_tensor("WoT", [EMB, EMB], BF16, kind="ExternalInput").ap(),
        nc.dram_tensor("bo", [1, EMB], BF16, kind="ExternalInput").ap(),
        nc.dram_tensor("sel", [65, 128], BF16, kind="ExternalInput").ap(),
        nc.dram_tensor("ones", [1, 128], BF16, kind="ExternalInput").ap(),
        nc.dram_tensor("Y", [N, EMB], F32, kind="ExternalOutput").ap(),
    )
    with tile.TileContext(nc, trace_sim=trace_sim) as tc:
        for rep in range(repeat):
            emit_body(tc, nc, aps, N, EMB, NH, rep)
    nc.compile()
    return nc


def host_consts(Wq, Wk, Wv, Wo, bo, NH=16):
    EMB = NH * 64
    bf = ml_dtypes.bfloat16

    A = np.asarray(Wq, np.float32).T @ np.asarray(Wk, np.float32)

    def blk2(B):
        out = np.zeros((128, 128), np.float32)
        out[0:64, 0:64] = B
        out[64:128, 64:128] = B
        return out

    # selector: row 0 -> output partitions 0..63, row 64 -> 64..127
    sel = np.zeros((65, 128), np.float32)
    sel[0, 0:64] = 1.0
    sel[64, 64:128] = 1.0
    return {
        "A2": blk2(A).astype(bf),
        "Wv2": blk2(np.asarray(Wv, np.float32).T).astype(bf),
        "WoT": np.ascontiguousarray(
            np.asarray(Wo, np.float32).T).astype(bf),
        "bo": np.asarray(bo, np.float32).reshape(1, EMB).astype(bf),
        "sel": sel.astype(bf),
        "ones": np.ones((1, 128), np.float32).astype(bf),
    }


def stage_x(X_core):
    """Convert one core's [N, EMB] fp32 activation slab to bf16."""
    return np.ascontiguousarray(
        np.asarray(X_core, np.float32).astype(ml_dtypes.bfloat16))


_NC_CACHE = {}


def kernel(X, Wq, Wk, Wv, Wo, bo):
    X = np.asarray(X, np.float32)
    B, N, EMB = X.shape
    NH = EMB // 64
    key = (N, EMB, NH, B)
    if key not in _NC_CACHE:
        _NC_CACHE[key] = build_program(N=N, EMB=EMB, NH=NH, n_cores=B)
    nc = _NC_CACHE[key]
    consts = host_consts(Wq, Wk, Wv, Wo, bo, NH=NH)
    in_maps = [dict(consts, X=stage_x(X[c])) for c in range(B)]
    res = run_bass_kernel_spmd(nc, in_maps, list(range(B)))
    return np.stack([res.results[c]["Y"] for c in range(B)], axis=0)


if __name__ == "__main__":
    rng = np.random.default_rng(0)
    B, N, EMB, NH = 8, 1024, 1024, 16
    X = rng.standard_normal((B, N, EMB), dtype=np.float32)
    Wq = (rng.standard_normal((64, 64), dtype=np.float32) / 8)
    Wk = (rng.standard_normal((64, 64), dtype=np.float32) / 8)
    Wv = (rng.standard_normal((64, 64), dtype=np.float32) / 8)
    Wo = (rng.standard_normal((EMB, EMB), dtype=np.float32) / 32)
    bo = np.zeros(EMB, np.float32)
    Y = kernel(X=X, Wq=Wq, Wk=Wk, Wv=Wv, Wo=Wo, bo=bo)
    print("OK", Y.shape, Y.dtype)
